# revision 1
# baseline (speedup 1.0000x reference)
"""GCN (2x GCNConv + mean-pool + linear) on 8 Trainium2 NeuronCores.

Single-launch fused kernel.  The per-launch dispatch overhead through the
PJRT tunnel (~86 ms fixed + ~0.1 ms/MB shipped) dominates the on-device
execution (~1 ms/layer), so v2 fuses both GCN layers, the pooling and the
final linear into ONE launch and strips the shipped inputs down to compact
per-core edge metadata (~1.2 MB/core):

  - x shard (own nodes, dinsq-prescaled, fp16) -> on-device AllGather ->
    on-device build of the [N, 128] layer-1 gather table (cols 4+ garbage,
    never read: matmuls only consume 0:4).
  - layer 1 runs the baseline message-passing scheme (SWDGE row gather +
    is_equal one-hot + PE scatter-add in PSUM), writes the prescaled h1
    shard; an AllGather produces each core's full [N, 128] fp16 layer-2
    table on device (no host round-trip, no replicated upload).
  - layer 2 ditto + mean-pool matmul + final linear; host sums the 8
    per-core [G, OUT] partials (pooling weights are built on device from
    per-node graph ids via is_equal×mult).
  - iota / identity matrices are built on device; gather indices ship as a
    single 16-partition wrap and are replicated to 128 partitions on device.

Normalization algebra (unchanged from baseline): with deg including the
self loop, the stored table is t_i = dinsq_i * relu(P_i) * dinsq_i where
P_i = W^T (sum_{e->i} t_src + t_i_own) + b*sqrtdeg_i, so the src-side
dinsq is pre-folded into the table and the dst-side dinsq commutes out of
the relu; the pooling weights carry the final dst dinsq * 1/cnt.
"""

import sys
from contextlib import ExitStack

for _p in ("/opt/trn_rl_repo",):
    if _p not in sys.path:
        sys.path.insert(0, _p)

import numpy as np

import concourse.bass as bass
import concourse.mybir as mybir
import concourse.tile as tile
from concourse import bacc
from concourse.bass_utils import run_bass_kernel_spmd
from concourse.library_config import mlp

FP16 = mybir.dt.float16
F32 = mybir.dt.float32
I16 = mybir.dt.int16
FP16_NP = np.float16


class Cfg:
    def __init__(self, N=100000, E=1600000, G=100, DIN=3, H=128, OUT=10,
                 NCORES=8, WT=8, SC_SIZE=25088):
        self.N, self.E, self.G = N, E, G
        self.DIN, self.H, self.OUT = DIN, H, OUT
        self.NCORES = NCORES
        assert N % NCORES == 0
        self.NPC = N // NCORES                      # nodes per core
        self.NT = (self.NPC + 127) // 128           # dst tiles per core
        self.LAST_VALID = self.NPC - (self.NT - 1) * 128
        self.WT = WT                                # tiles per wave
        self.NW = (self.NT + WT - 1) // WT
        assert SC_SIZE <= 32768
        self.SC_SIZE = SC_SIZE                      # src chunk rows (int16 idx)
        self.NSC = (N + SC_SIZE - 1) // SC_SIZE


FULL = Cfg()


# --------------------------------------------------------------------------
# host preprocessing
# --------------------------------------------------------------------------

def preprocess(cfg, x, edge_index, batch):
    N, G, NC = cfg.N, cfg.G, cfg.NCORES
    NPC, NT, NSC, WT = cfg.NPC, cfg.NT, cfg.NSC, cfg.WT
    src = np.asarray(edge_index[0], dtype=np.int64)
    dst = np.asarray(edge_index[1], dtype=np.int64)
    batch = np.asarray(batch, dtype=np.int64)
    x = np.asarray(x, dtype=np.float32)

    deg = (np.bincount(dst, minlength=N) + 1.0).astype(np.float32)
    dinsq = (1.0 / np.sqrt(deg)).astype(np.float32)
    sqrtdeg = np.sqrt(deg).astype(np.float32)
    cnt = np.bincount(batch, minlength=G).astype(np.float32)
    invcnt = (1.0 / np.maximum(cnt, 1.0)).astype(np.float32)

    # prescaled node features, padded to 4 cols, node-major fp16; padded to
    # a whole number of src chunks so the on-device table build can load
    # contiguous per-partition stripes
    NXP = cfg.NSC * cfg.SC_SIZE
    xs = np.zeros((NXP, 4), dtype=FP16_NP)
    xs[:N, :cfg.DIN] = (x * dinsq[:, None]).astype(FP16_NP)

    core = dst // NPC
    dst_local = dst - core * NPC
    tl = dst_local >> 7
    sc = src // cfg.SC_SIZE
    key = (core * NT + tl) * NSC + sc
    order = np.argsort(key, kind="stable")
    key_s = key[order]
    src_s = src[order]
    sc_s = sc[order]

    counts = np.bincount(key_s, minlength=NC * NT * NSC).reshape(NC, NT, NSC)
    # per-(tile, srcchunk) slot count, padded to 32 and uniform across cores
    GRAIN = 32
    P = ((counts.max(axis=0) + GRAIN - 1) // GRAIN * GRAIN).astype(np.int64)

    waves = [list(range(w * WT, min((w + 1) * WT, NT))) for w in range(cfg.NW)]
    # group = (wave, srcchunk): tiles' slot ranges concatenated, chunked by 128
    slot_base = np.zeros((NT, NSC), dtype=np.int64)   # global slot index
    gmeta = []     # per wave: per s: (idx_col0, nidx, msgcol0, nch)
    wmms = []      # per wave: ordered list of (mcol, j_in_wave, width, gcol)
    pos = 0        # global chunk counter
    SENT = 16384.0
    for w, wtiles in enumerate(waves):
        wmeta = []
        wave_chunk0 = pos
        mms = []
        for s in range(NSC):
            c0 = pos
            off = 0     # slot offset within group
            spans = []  # (t, slot_lo, slot_hi) within group
            for t in wtiles:
                slot_base[t, s] = c0 * 128 + off
                if P[t, s]:
                    spans.append((t, off, off + int(P[t, s])))
                off += int(P[t, s])
            nch = (off + 127) // 128
            for k in range(nch):
                lo, hi = k * 128, (k + 1) * 128
                sp = [t for t, a, b in spans if a < hi and b > lo]
                if not sp:
                    continue
                j0, j1 = sp[0] - wtiles[0], sp[-1] - wtiles[0]
                jj = j0
                while jj <= j1:          # split windows at psum-bank groups
                    je = min(j1, (jj // 4) * 4 + 3)
                    mms.append((c0 - wave_chunk0 + k, jj, je - jj + 1, c0 + k))
                    jj = je + 1
            pos += nch
            wmeta.append((c0 * 8, nch * 128, c0 - wave_chunk0, nch))
        gmeta.append(wmeta)
        wmms.append(mms)
    TOTCH = pos
    CW = max(sum(gmeta[w][s][3] for s in range(NSC))
             for w in range(cfg.NW))

    # scatter edge data into padded per-core arrays
    idx_all = np.zeros((NC, TOTCH * 128), dtype=np.int16)
    dstw_all = np.full((NC, TOTCH * 128), SENT, dtype=FP16_NP)
    bstart = np.zeros(NC * NT * NSC, dtype=np.int64)
    cflat = counts.reshape(-1)
    bstart[1:] = np.cumsum(cflat)[:-1]
    rank = np.arange(len(key_s)) - bstart[key_s]
    ccore = key_s // (NT * NSC)
    rem = key_s % (NT * NSC)
    dest = slot_base.reshape(-1)[rem] + rank
    idx_all[ccore, dest] = (src_s - sc_s * cfg.SC_SIZE).astype(np.int16)
    # dst index relative to the wave's first tile
    wavebase = (tl[order] // WT) * WT * 128
    dstw_all[ccore, dest] = (dst_local[order] - wavebase).astype(FP16_NP)

    # single 16-partition wrap (replicated to 128 partitions on device)
    idx16 = np.ascontiguousarray(
        idx_all.reshape(NC, TOTCH * 8, 16).transpose(0, 2, 1))
    dstw = np.ascontiguousarray(
        dstw_all.reshape(NC, TOTCH, 128).transpose(0, 2, 1))

    # per-core per-node columns (padded to NT*128)
    NPAD = NT * 128
    invdeg_col = np.zeros((NC, 128, NT), dtype=np.float32)
    sqrtdeg_row = np.ones((NC, 1, NPAD), dtype=np.float32)
    bcol = np.full((NC, 128, NT), SENT, dtype=np.float32)  # graph id per node
    wvc = np.zeros((NC, 128, NT), dtype=np.float32)      # dinsq*invcnt
    xss = np.zeros((NC, NPC, 4), dtype=FP16_NP)          # own prescaled x rows
    for c in range(NC):
        idx = np.arange(NPAD) + c * NPC
        valid = np.arange(NPAD) < NPC
        idx = np.where(valid, idx, 0)
        iv = np.where(valid, (dinsq * dinsq)[idx], 1.0).astype(np.float32)
        invdeg_col[c] = iv.reshape(NT, 128).T
        sqrtdeg_row[c, 0] = np.where(valid, sqrtdeg[idx], 1.0)
        bc = np.where(valid, batch[idx].astype(np.float32), SENT)
        bcol[c] = bc.reshape(NT, 128).T
        wv = np.where(valid, dinsq[idx] * invcnt[batch[idx]], 0.0)
        wvc[c] = wv.astype(np.float32).reshape(NT, 128).T
        xss[c] = xs[c * NPC:(c + 1) * NPC]

    return dict(
        xs=xs, xss=xss, idx16=idx16, dstw=dstw,
        invdeg_col=invdeg_col, sqrtdeg_row=sqrtdeg_row,
        bcol=bcol, wvc=wvc,
        waves=waves, gmeta=gmeta, wmms=wmms,
        TOTCH=TOTCH, CW=CW, deg=deg,
    )


# --------------------------------------------------------------------------
# fused kernel builder
# --------------------------------------------------------------------------

def build_fused(cfg, meta, has_b1, has_b2):
    N, G, OUT = cfg.N, cfg.G, cfg.OUT
    NT, NSC, WT, NPC = cfg.NT, cfg.NSC, cfg.WT, cfg.NPC
    TOTCH, CW = meta["TOTCH"], meta["CW"]
    waves, gmeta, wmms = meta["waves"], meta["gmeta"], meta["wmms"]
    NPAD = NT * 128
    NC = cfg.NCORES
    GROUPS = [list(range(NC))]

    assert cfg.SC_SIZE % 128 == 0
    NXP = NSC * cfg.SC_SIZE          # padded node count for the x table

    nc = bacc.Bacc("TRN2", target_bir_lowering=False, debug=False,
                   num_devices=NC, num_swdge_queues=4,
                   dynamic_dma_scratch_size=32768)
    xsr_d = nc.dram_tensor("xsr", [NXP, 4], FP16, kind="ExternalInput")
    xss_d = nc.dram_tensor("xss", [NPC, 4], FP16, kind="ExternalInput")
    idx_d = nc.dram_tensor("idx", [16, TOTCH * 8], I16, kind="ExternalInput")
    dstw_d = nc.dram_tensor("dstw", [128, TOTCH], FP16, kind="ExternalInput")
    # scalar operands of is_equal must be f32 on DVE; dstw/bcol are converted
    # (or shipped) as f32 accordingly
    w1_d = nc.dram_tensor("w1", [4, 128], F32, kind="ExternalInput")
    w2_d = nc.dram_tensor("w2", [128, 128], F32, kind="ExternalInput")
    wl_d = nc.dram_tensor("wl", [128, OUT], F32, kind="ExternalInput")
    if has_b1:
        b1_d = nc.dram_tensor("b1r", [1, 128], F32, kind="ExternalInput")
    if has_b2:
        b2_d = nc.dram_tensor("b2r", [1, 128], F32, kind="ExternalInput")
    if has_b1 or has_b2:
        sqd_d = nc.dram_tensor("sqd", [1, NPAD], F32, kind="ExternalInput")
    ivd_d = nc.dram_tensor("ivd", [128, NT], F32, kind="ExternalInput")
    bcol_d = nc.dram_tensor("bcol", [128, NT], F32, kind="ExternalInput")
    wvc_d = nc.dram_tensor("wvc", [128, NT], F32, kind="ExternalInput")
    out_d = nc.dram_tensor("out", [G, OUT], F32, kind="ExternalOutput")

    relu = mybir.ActivationFunctionType.Relu
    copy_fn = mybir.ActivationFunctionType.Copy
    iseq = mybir.AluOpType.is_equal

    with tile.TileContext(nc) as tc:
        nc.gpsimd.load_library(mlp)
        with ExitStack() as ctx:
            const = ctx.enter_context(tc.tile_pool(name="const", bufs=1))
            dram = ctx.enter_context(tc.tile_pool(name="dram", bufs=1,
                                                  space="DRAM"))

            # ---- DRAM internals. The layer-1 gather table is split per
            # src-chunk so wave gathers start as soon as their chunk is built.
            xpad_ts = [dram.tile([cfg.SC_SIZE, 128], FP16, name=f"xpad{s}")
                       for s in range(NSC)]
            h1b_t = dram.tile([NPC, 128], FP16)
            h1full_t = dram.tile([N, 128], FP16, addr_space="Shared")

            # ---- phase 0b: build the layer-1 gather tables from the
            # replicated prescaled x; issued before the constant loads so the
            # first wave's gathers unblock as early as possible. Table row ==
            # node id within each chunk: partition p holds the contiguous
            # node stripe [s*SC + p*PCH, +PCH), so both the x load (one
            # descriptor per partition) and the table write (2KB row runs)
            # stay coarse. Only cols 0:4 are ever read downstream; 4:128
            # stay garbage.
            xbp = ctx.enter_context(tc.tile_pool(name="xb", bufs=2))
            zsp = ctx.enter_context(tc.tile_pool(name="zs", bufs=2))
            PCH = cfg.SC_SIZE // 128     # nodes per partition stripe
            BW = 49                      # 4 big DMAs per chunk (196 = 4*49)
            for s in range(NSC):
                xsc = xbp.tile([128, PCH * 4], FP16, tag="xsc")
                nc.sync.dma_start(
                    xsc[:],
                    xsr_d[s * cfg.SC_SIZE:(s + 1) * cfg.SC_SIZE, :]
                    .rearrange("(p t) f -> p (t f)", p=128))
                tab_view = xpad_ts[s][:, :].rearrange(
                    "(p t) f -> p t f", t=PCH)
                for bi, b0 in enumerate(range(0, PCH, BW)):
                    nb = min(BW, PCH - b0)
                    zst = zsp.tile([128, BW, 128], FP16, tag="zst")
                    nc.vector.tensor_copy(
                        zst[:, 0:nb, 0:4],
                        xsc[:, b0 * 4:(b0 + nb) * 4]
                        .rearrange("p (t f) -> p t f", f=4))
                    (nc.sync if bi % 2 == 0 else nc.scalar).dma_start(
                        tab_view[:, b0:b0 + nb, :], zst[:, 0:nb, :])

            # ---- constants
            # gather indices, replicated to all 8 Q7 stripes; wave 0's
            # columns are loaded first so its gathers unblock early, and
            # their issue is spread across engines (each dma_start costs
            # ~5us of issue latency on its issuing engine's queue)
            idx_t = const.tile([128, TOTCH * 8], I16)
            w0cols = sum(gmeta[0][s][3] for s in range(NSC)) * 8
            issuers = [nc.sync, nc.scalar]
            for k in range(8):
                issuers[k % 2].dma_start(idx_t[16 * k:16 * k + 16, 0:w0cols],
                                         idx_d[:, 0:w0cols])
            for k in range(8):
                nc.sync.dma_start(idx_t[16 * k:16 * k + 16, w0cols:],
                                  idx_d[:, w0cols:])
            dstw_t = const.tile([128, TOTCH], F32)
            iota16 = const.tile([128, WT * 128], I16)
            nc.gpsimd.iota(iota16[:], [[1, WT * 128]], channel_multiplier=0)
            iota_t = const.tile([128, WT * 128], FP16)
            nc.any.tensor_copy(iota_t[:], iota16[:])
            ig16 = const.tile([128, 128], I16)
            nc.gpsimd.iota(ig16[:], [[1, 128]], channel_multiplier=0)
            pid16 = const.tile([128, 1], I16)
            nc.gpsimd.iota(pid16[:], [[1, 1]], channel_multiplier=1)
            pidf = const.tile([128, 1], F32)
            nc.any.tensor_copy(pidf[:], pid16[:])
            ident_t = const.tile([128, 128], F32)
            nc.vector.tensor_scalar(ident_t[:], ig16[:], pidf[:], None, iseq)
            identb_t = const.tile([128, 128], FP16)
            nc.any.tensor_copy(identb_t[:], ident_t[:])
            iotag_t = const.tile([128, 128], FP16)
            nc.any.tensor_copy(iotag_t[:], ig16[:])
            zc_t = const.tile([1, 512], FP16)
            nc.vector.memset(zc_t[:], 0.0)
            w1_t = const.tile([4, 128], F32)
            nc.sync.dma_start(w1_t[:], w1_d[:])
            w2_t = const.tile([128, 128], F32)
            nc.sync.dma_start(w2_t[:], w2_d[:])
            wl_t = const.tile([128, OUT], F32)
            nc.sync.dma_start(wl_t[:], wl_d[:])
            if has_b1:
                b1r_t = const.tile([1, 128], F32)
                nc.sync.dma_start(b1r_t[:], b1_d[:])
            if has_b2:
                b2r_t = const.tile([1, 128], F32)
                nc.sync.dma_start(b2r_t[:], b2_d[:])
            if has_b1 or has_b2:
                sqd_t = const.tile([1, NPAD], F32)
                nc.sync.dma_start(sqd_t[:], sqd_d[:])
            ivd_t = const.tile([128, NT], F32)
            nc.sync.dma_start(ivd_t[:], ivd_d[:])
            bcol_t = const.tile([128, NT], F32)
            nc.sync.dma_start(bcol_t[:], bcol_d[:])
            with tc.tile_pool(name="dsw", bufs=1) as dswp:
                dstwh = dswp.tile([128, TOTCH], FP16)
                nc.sync.dma_start(dstwh[:], dstw_d[:])
                nc.any.tensor_copy(dstw_t[:], dstwh[:])
            wvc_t = const.tile([128, NT], F32)
            nc.sync.dma_start(wvc_t[:], wvc_d[:])
            # own x rows in tile layout [p, (t f)]
            NTF = NPC // 128
            TTAIL = NPC - NTF * 128
            xso_t = const.tile([128, NT * 4], FP16)
            nc.sync.dma_start(
                xso_t[:, 0:NTF * 4].rearrange("p (t f) -> p t f", f=4),
                xss_d[0:NTF * 128, :].rearrange("(t p) f -> p t f", p=128))
            if TTAIL:
                nc.sync.dma_start(xso_t[0:TTAIL, NTF * 4:NTF * 4 + 4],
                                  xss_d[NTF * 128:NPC, :])
            # resident prescaled h1 (layer-1 output, node within tile on
            # partitions, (tile, feat) on free dim)
            h1sb = const.tile([128, NPAD], FP16)

            # ---- shared pools for both layers (a single set avoids a pool
            # release/alloc barrier at the collective between the layers)
            msgp = ctx.enter_context(tc.tile_pool(name="msg", bufs=2))
            ohp = ctx.enter_context(tc.tile_pool(name="oh", bufs=4))
            asbp = ctx.enter_context(tc.tile_pool(name="asb", bufs=2))
            rlp = ctx.enter_context(tc.tile_pool(name="rl", bufs=2))
            stp = ctx.enter_context(tc.tile_pool(name="st", bufs=2))
            aggp = ctx.enter_context(tc.tile_pool(name="agg", bufs=4,
                                                  space="PSUM"))
            p2p = ctx.enter_context(tc.tile_pool(name="p2", bufs=1,
                                                 space="PSUM"))
            trp = ctx.enter_context(tc.tile_pool(name="tr", bufs=2,
                                                 space="PSUM"))
            pwp = ctx.enter_context(tc.tile_pool(name="pw", bufs=3))
            plp = ctx.enter_context(tc.tile_pool(name="pl", bufs=1,
                                                 space="PSUM"))

            # ---- the two GCN layers
            def layer(lnum, tab_of, KIN, w_t, brow_t, has_bias):
                gq = 0
                CALL_CHUNKS = 48
                if True:
                    if lnum == 2:
                        pooled_ps = plp.tile([128, G], F32)

                    for w, wtiles in enumerate(waves):
                        msg = msgp.tile([128, CW, 128], FP16, tag="msg")
                        for s in range(NSC):
                            icol0, nidx, mcol0, nch = gmeta[w][s]
                            if nidx == 0:
                                continue
                            for cb in range(0, nch, CALL_CHUNKS):
                                ce = min(cb + CALL_CHUNKS, nch)
                                ni = (ce - cb) * 128
                                nc.gpsimd.dma_gather(
                                    msg[:, mcol0 + cb:mcol0 + ce, :],
                                    tab_of(s),
                                    idx_t[:, icol0 + cb * 8:
                                          icol0 + cb * 8 + ni // 16],
                                    ni, ni, 128,
                                    single_packet=False,
                                    queue_num=gq % 4,
                                )
                                gq += 1
                        # zero each psum bank with a full-width PE matmul; all
                        # chunk matmuls then accumulate in any order
                        mms = wmms[w]
                        aggs = [aggp.tile([KIN, 512], F32, tag="agg",
                                          name=f"agg{lnum}_w{w}_{h}")
                                for h in range((len(wtiles) + 3) // 4)]
                        for agg in aggs:
                            nc.tensor.matmul(agg[:], zc_t[0:1, 0:KIN],
                                             zc_t[0:1, 0:512],
                                             start=True, stop=False,
                                             skip_group_check=True)
                        for mcol, j0, wid, gcol in mms:
                            oh = ohp.tile([128, wid * 128], FP16, tag="oh")
                            nc.vector.tensor_scalar(
                                oh[:], iota_t[:, j0 * 128:(j0 + wid) * 128],
                                dstw_t[:, gcol:gcol + 1], None, iseq)
                            agg = aggs[j0 // 4]
                            psl = agg[:, (j0 % 4) * 128:(j0 % 4 + wid) * 128]
                            nc.tensor.matmul(
                                psl, msg[:, mcol, 0:KIN], oh[:],
                                start=False, stop=False,
                                skip_group_check=True)
                        # self-loop rows close each bank's accumulation group
                        for j, t in enumerate(wtiles):
                            rows = min(128, NPC - t * 128)
                            if lnum == 1:
                                own_ap = xso_t[0:rows, t * 4:t * 4 + 4]
                            else:
                                own_ap = h1sb[0:rows, t * 128:t * 128 + KIN]
                            psl = aggs[j // 4][:, (j % 4) * 128:
                                               (j % 4) * 128 + 128]
                            nc.tensor.matmul(
                                psl, own_ap, identb_t[0:rows, :],
                                start=False,
                                stop=(j % 4 == 3 or j == len(wtiles) - 1),
                                skip_group_check=True)
                        for j, t in enumerate(wtiles):
                            psl = aggs[j // 4][:, (j % 4) * 128:
                                               (j % 4) * 128 + 128]
                            agg_sb = asbp.tile([KIN, 128], F32, tag="asb")
                            nc.scalar.activation(agg_sb[:], psl, copy_fn)
                            p2 = p2p.tile([128, 128], F32, tag="p2")
                            nc.tensor.matmul(p2[:], w_t[:], agg_sb[:],
                                             start=True, stop=not has_bias)
                            if has_bias:
                                nc.tensor.matmul(
                                    p2[:], brow_t[:],
                                    sqd_t[0:1, t * 128:t * 128 + 128],
                                    start=False, stop=True)
                            relu_sb = rlp.tile([128, 128], F32, tag="rl")
                            nc.scalar.activation(relu_sb[:], p2[:], relu)
                            tnm = trp.tile([128, 128], F32, tag="tr")
                            nc.tensor.transpose(tnm[:], relu_sb[:], ident_t[:])
                            if lnum == 1:
                                nc.scalar.activation(
                                    h1sb[:, t * 128:t * 128 + 128], tnm[:],
                                    copy_fn, scale=ivd_t[:, t:t + 1])
                            else:
                                tnm_sb = stp.tile([128, 128], F32, tag="tnm")
                                nc.scalar.activation(tnm_sb[:], tnm[:],
                                                     copy_fn)
                                pw_t = pwp.tile([128, G], F32, tag="pw")
                                nc.vector.tensor_scalar(
                                    pw_t[:], iotag_t[:, 0:G],
                                    bcol_t[:, t:t + 1], wvc_t[:, t:t + 1],
                                    iseq, mybir.AluOpType.mult)
                                nc.tensor.matmul(
                                    pooled_ps[:], tnm_sb[:], pw_t[:],
                                    start=(t == 0), stop=(t == NT - 1),
                                    skip_group_check=True)
                        if lnum == 1:
                            # bounce this wave's rows to DRAM for the gather
                            base = wtiles[0] * 128
                            nfull = sum(1 for t in wtiles
                                        if (t + 1) * 128 <= NPC)
                            if nfull:
                                nc.sync.dma_start(
                                    h1b_t[base:base + nfull * 128, :]
                                    .rearrange("(j p) f -> p j f", p=128),
                                    h1sb[:, base:base + nfull * 128]
                                    .rearrange("p (j f) -> p j f", f=128))
                            for t in wtiles:
                                if (t + 1) * 128 <= NPC:
                                    continue
                                rows = NPC - t * 128
                                if rows > 0:
                                    nc.sync.dma_start(
                                        h1b_t[t * 128:t * 128 + rows, :],
                                        h1sb[0:rows, t * 128:(t + 1) * 128])

                    if lnum == 2:
                        pooled_sb = const.tile([128, G], F32)
                        nc.any.tensor_copy(pooled_sb[:], pooled_ps[:])
                        outp = p2p.tile([128, 128], F32, tag="p2")
                        nc.tensor.matmul(outp[0:G, 0:OUT], pooled_sb[:],
                                         wl_t[:], start=True, stop=True,
                                         skip_group_check=True)
                        out_sb = const.tile([G, OUT], F32)
                        nc.any.tensor_copy(out_sb[:], outp[0:G, 0:OUT])
                        nc.sync.dma_start(out_d[:], out_sb[:])

            layer(1, lambda s: xpad_ts[s][:, :], 4, w1_t,
                  b1r_t if has_b1 else None, has_b1)

            nc.gpsimd.collective_compute(
                "AllGather", mybir.AluOpType.bypass, replica_groups=GROUPS,
                ins=[h1b_t[:].opt()], outs=[h1full_t[:].opt()])

            layer(2, lambda s: h1full_t[s * cfg.SC_SIZE:
                                        min(N, (s + 1) * cfg.SC_SIZE), :],
                  128, w2_t, b2r_t if has_b2 else None, has_b2)

    nc.compile()
    return nc


# --------------------------------------------------------------------------
# driver
# --------------------------------------------------------------------------

def _run(cfg, meta, W1, b1, W2, b2, Wl, bl, runner):
    NC = cfg.NCORES
    has_b1 = bool(np.any(np.asarray(b1)))
    has_b2 = bool(np.any(np.asarray(b2)))

    assert cfg.DIN <= 4
    W1p = np.zeros((4, 128), dtype=np.float32)
    W1p[:cfg.DIN] = np.asarray(W1, dtype=np.float32)

    nc = build_fused(cfg, meta, has_b1, has_b2)
    in_maps = []
    for c in range(NC):
        m = dict(
            xsr=meta["xs"], xss=meta["xss"][c],
            idx=meta["idx16"][c], dstw=meta["dstw"][c],
            w1=W1p, w2=np.asarray(W2, np.float32),
            wl=np.asarray(Wl, np.float32),
            ivd=meta["invdeg_col"][c], bcol=meta["bcol"][c],
            wvc=meta["wvc"][c],
        )
        if has_b1:
            m["b1r"] = np.asarray(b1, np.float32).reshape(1, 128)
        if has_b2:
            m["b2r"] = np.asarray(b2, np.float32).reshape(1, 128)
        if has_b1 or has_b2:
            m["sqd"] = meta["sqrtdeg_row"][c]
        in_maps.append(m)
    res = runner(nc, in_maps)
    total = np.sum([res[c]["out"] for c in range(NC)], axis=0)
    return (total + np.asarray(bl, np.float32)[None, :]).astype(np.float32)


def _hw_runner(nc, in_maps):
    core_ids = list(range(len(in_maps)))
    try:
        res = run_bass_kernel_spmd(nc, in_maps, core_ids=core_ids)
    except Exception:
        # one retry for transient tunnel/device failures
        res = run_bass_kernel_spmd(nc, in_maps, core_ids=core_ids)
    return res.results


def kernel(x, edge_index, batch, W1, b1, W2, b2, Wl, bl):
    cfg = FULL
    meta = preprocess(cfg, x, edge_index, batch)
    return _run(cfg, meta, W1, b1, W2, b2, Wl, bl, _hw_runner)



# revision 2
# speedup vs baseline: 1.0729x; 1.0729x over previous
"""GCN (2x GCNConv + mean-pool + linear) on 8 Trainium2 NeuronCores.

On-device time is bound by the layer-2 per-edge dma_gather (256B rows,
~55GB/s SWDGE ceiling), so v4 makes everything else nearly free and
overlaps it under the gather window:

  - Nodes are assigned to (tile, partition) slots per core by descending
    in-degree (the node->slot map is free to choose; all per-node tables
    are host-built in slot order). Layer-1 messages are then shipped as a
    dense dst-major table [128, wave, tile, feat, k] so the whole layer-1
    aggregation is a single DVE tensor_reduce per wave: no gather, no
    one-hots, no scatter matmuls; the self-loop is just one more entry.
  - Layer-1 epilogue is wave-batched: per-tile PE transposes into PSUM,
    one W1 matmul per 512-column half, one relu per half.
  - Layer-1 output is AllGather'd in 4 wave-aligned quarters overlapping
    layer-1; layer-2 gathers (tile-pure 128-slot chunks) start as soon as
    their quarter table lands and run continuously on all 4 SWDGE queues
    into wave-sized message tiles.
  - Layer-2 scatter one-hots are built in ONE DVE tensor_tensor(is_equal)
    with broadcast APs per (wave, quarter) group.
"""

import sys
from contextlib import ExitStack

for _p in ("/opt/trn_rl_repo",):
    if _p not in sys.path:
        sys.path.insert(0, _p)

import numpy as np

import concourse.bass as bass
import concourse.mybir as mybir
import concourse.tile as tile
from concourse import bacc
from concourse.bass_utils import run_bass_kernel_spmd
from concourse.library_config import mlp

FP16 = mybir.dt.float16
F32 = mybir.dt.float32
I16 = mybir.dt.int16
FP16_NP = np.float16
SENT = 16384.0


class Cfg:
    def __init__(self, N=100000, E=1600000, G=100, DIN=3, H=128, OUT=10,
                 NCORES=8, WT=8):
        self.N, self.E, self.G = N, E, G
        self.DIN, self.H, self.OUT = DIN, H, OUT
        self.NCORES = NCORES
        assert N % NCORES == 0
        self.NPC = N // NCORES
        self.NT = (self.NPC + 127) // 128
        self.WT = WT
        self.NW = (self.NT + WT - 1) // WT
        self.QWAVES = [(0, 3), (3, 6), (6, 9), (9, self.NW)]
        self.QSTART = [w0 * WT for (w0, _) in self.QWAVES]
        qend = [min(w1 * WT, self.NT) for (_, w1) in self.QWAVES]
        self.QROWS = [min(e * 128, self.NPC) - s * 128
                      for s, e in zip(self.QSTART, qend)]
        assert sum(self.QROWS) == self.NPC
        assert all(r * NCORES < 32768 for r in self.QROWS)


FULL = Cfg()


# --------------------------------------------------------------------------
# host preprocessing
# --------------------------------------------------------------------------

def preprocess(cfg, x, edge_index, batch):
    N, G, NC = cfg.N, cfg.G, cfg.NCORES
    NPC, NT, WT, NW = cfg.NPC, cfg.NT, cfg.WT, cfg.NW
    src = np.asarray(edge_index[0], dtype=np.int64)
    dst = np.asarray(edge_index[1], dtype=np.int64)
    batch = np.asarray(batch, dtype=np.int64)
    x = np.asarray(x, dtype=np.float32)
    E = src.shape[0]

    deg = (np.bincount(dst, minlength=N) + 1.0).astype(np.float32)
    dinsq = (1.0 / np.sqrt(deg)).astype(np.float32)
    sqrtdeg = np.sqrt(deg).astype(np.float32)
    cnt = np.bincount(batch, minlength=G).astype(np.float32)
    invcnt = (1.0 / np.maximum(cnt, 1.0)).astype(np.float32)

    xs16 = np.zeros((N, 4), dtype=FP16_NP)
    xs16[:, :cfg.DIN] = (x * dinsq[:, None]).astype(FP16_NP)

    # ---- per-core node -> slot map (identity; degree sorting concentrates
    # L2 edges into early waves and blows up the message tiles)
    order = np.tile(np.arange(NPC), (NC, 1))           # slot -> local node
    slotof = order                                     # local node -> slot

    core = dst // NPC
    dstl = dst - core * NPC
    s_dst = slotof[core, dstl]
    t = s_dst >> 7
    dit = (s_dst & 127).astype(np.float32)

    # ---- L1 dst-major message table with per-wave entry budget K_w
    dcnt = np.zeros((NC, NT * 128), np.int64)          # per-slot in-degree
    np.add.at(dcnt, (core, s_dst), 1)
    K_w = []
    for w in range(NW):
        lo, hi = w * WT * 128, min((w + 1) * WT, NT) * 128
        K_w.append(int(dcnt[:, lo:hi].max()) + 1)      # +1 self entry
    mb1 = np.zeros(NW + 1, np.int64)                   # col base per wave
    for w in range(NW):
        nw_ = min((w + 1) * WT, NT) - w * WT
        mb1[w + 1] = mb1[w] + nw_ * 4 * K_w[w]
    M1COLS = int(mb1[NW])

    msg1 = np.zeros((NC, 128, M1COLS), dtype=FP16_NP)
    # edge entries: k = rank within (core, dst-slot)
    okey = core * (NT * 128) + s_dst
    oo = np.argsort(okey, kind="stable")
    bs = np.zeros(NC * NT * 128, np.int64)
    bs[1:] = np.cumsum(dcnt.reshape(-1))[:-1]
    krank = np.arange(E) - bs[okey[oo]]
    s_o, c_o = s_dst[oo], core[oo]
    t_o = s_o >> 7
    w_o = t_o // WT
    j_o = t_o - w_o * WT
    p_o = s_o & 127
    kw_o = np.array(K_w)[w_o]
    colbase = mb1[w_o] + (j_o * 4) * kw_o + krank
    vals = xs16[src[oo]]
    for f in range(4):
        msg1[c_o, p_o, colbase + f * kw_o] = vals[:, f]
    # self entries at k = dcnt
    for c in range(NC):
        sarr = np.arange(NT * 128)
        valid = sarr < NPC
        sv = sarr[valid]
        tv = sv >> 7
        wv = tv // WT
        jv = tv - wv * WT
        pv = sv & 127
        kwv = np.array(K_w)[wv]
        cb = mb1[wv] + (jv * 4) * kwv + dcnt[c, sv]
        xvals = xs16[c * NPC + order[c][sv]]
        for f in range(4):
            msg1[c, pv, cb + f * kwv] = xvals[:, f]

    # ---- L2 slot layout: tile-pure chunks grouped (wave, quarter, tile)
    QROWS, QSTART = np.array(cfg.QROWS), np.array(cfg.QSTART)
    srcc = src // NPC
    srcl = src - srcc * NPC
    s_src = slotof[srcc, srcl]
    srct = s_src >> 7
    srcq = np.searchsorted(np.cumsum(QROWS), s_src, side="right")
    toff = srcc * QROWS[srcq] + (s_src - QSTART[srcq] * 128)
    assert toff.max() < 32768

    # chunks are tile-PAIR-pure: each 128-slot chunk targets one pair of
    # adjacent tiles (one-hot window 256, one matmul per chunk)
    NPR = (NT + 1) // 2
    pr = t >> 1
    dpr = (s_dst & 255).astype(np.float32)     # dst rel to pair start
    cnt2 = np.zeros((NC, NPR, 4), np.int64)
    np.add.at(cnt2, (core, pr, srcq), 1)
    nch2 = (cnt2.max(axis=0) + 127) // 128     # [NPR, 4]
    cb2 = np.zeros((NPR, 4), np.int64)
    pos = 0
    wq_meta = []
    for w in range(NW):
        wpairs = range(w * WT // 2, (min((w + 1) * WT, NT) + 1) // 2)
        wm = []
        for q in range(4):
            c0 = pos
            for pp in wpairs:
                cb2[pp, q] = pos
                pos += int(nch2[pp, q])
            wm.append((c0, pos - c0))
        wq_meta.append(wm)
    TOTCH2 = pos

    key2 = (core * NPR + pr) * 4 + srcq
    order2 = np.argsort(key2, kind="stable")
    b2s = np.zeros(NC * NPR * 4, np.int64)
    b2s[1:] = np.cumsum(cnt2.reshape(-1))[:-1]
    rank2 = np.arange(E) - b2s[key2[order2]]
    slot2 = cb2[pr[order2], srcq[order2]] * 128 + rank2
    c_o2 = core[order2]
    p2_, cc2 = slot2 % 128, slot2 // 128

    idxf = np.zeros((NC, TOTCH2 * 128), dtype=np.int16)
    dstw2 = np.full((NC, 128, TOTCH2), SENT, dtype=FP16_NP)
    idxf[c_o2, slot2] = toff[order2].astype(np.int16)
    dstw2[c_o2, p2_, cc2] = dpr[order2].astype(FP16_NP)
    idx16 = np.ascontiguousarray(
        idxf.reshape(NC, TOTCH2 * 8, 16).transpose(0, 2, 1))

    # ---- per-slot epilogue columns (slot order!)
    NPAD = NT * 128
    sarr = np.arange(NPAD)
    valid = sarr < NPC
    sv = np.where(valid, sarr, 0)
    ivd = np.zeros((NC, 128, NT), np.float32)
    bcolv = np.zeros((NC, 128, NT), np.float32)
    wvc = np.zeros((NC, 128, NT), np.float32)
    sqd = np.ones((NC, 1, NPAD), np.float32)
    for c in range(NC):
        g = c * NPC + order[c][sv]
        ivd[c] = np.where(valid, dinsq[g] ** 2, 1.0).reshape(NT, 128).T
        bcolv[c] = np.where(valid, batch[g].astype(np.float32),
                            SENT).reshape(NT, 128).T
        wvc[c] = np.where(valid, dinsq[g] * invcnt[batch[g]],
                          0.0).reshape(NT, 128).T
        sqd[c, 0] = np.where(valid, sqrtdeg[g], 1.0)

    CW2 = max(sum(n for (_, n) in wm) for wm in wq_meta)
    CWQ = max(n for wm in wq_meta for (_, n) in wm)

    return dict(
        msg1=msg1, idx16=idx16, dstw2=dstw2,
        ivd=ivd, bcol=bcolv, wvc=wvc, sqd=sqd,
        nch2=nch2, cb2=cb2, TOTCH2=TOTCH2, wq_meta=wq_meta,
        K_w=K_w, mb1=mb1, M1COLS=M1COLS, CW2=CW2, CWQ=CWQ, deg=deg,
    )


# --------------------------------------------------------------------------
# fused kernel builder
# --------------------------------------------------------------------------

def build_fused(cfg, meta, has_b1, has_b2):
    N, G, OUT = cfg.N, cfg.G, cfg.OUT
    NT, WT, NPC, NW = cfg.NT, cfg.WT, cfg.NPC, cfg.NW
    NPAD = NT * 128
    NC = cfg.NCORES
    GROUPS = [list(range(NC))]
    TOTCH2 = meta["TOTCH2"]
    nch2, cb2 = meta["nch2"], meta["cb2"]
    wq_meta = meta["wq_meta"]
    K_w, mb1, M1COLS = meta["K_w"], meta["mb1"], meta["M1COLS"]
    CW2, CWQ = meta["CW2"], meta["CWQ"]
    QROWS, QWAVES = cfg.QROWS, cfg.QWAVES
    M1WMAX = max(mb1[w + 1] - mb1[w] for w in range(NW))

    nc = bacc.Bacc("TRN2", target_bir_lowering=False, debug=False,
                   num_devices=NC, num_swdge_queues=4,
                   dynamic_dma_scratch_size=24576)
    msg1_d = nc.dram_tensor("msg1", [128, M1COLS], FP16,
                            kind="ExternalInput")
    idx_d = nc.dram_tensor("idx", [16, TOTCH2 * 8], I16,
                           kind="ExternalInput")
    dstw2_d = nc.dram_tensor("dstw2", [128, TOTCH2], FP16,
                             kind="ExternalInput")
    w1_d = nc.dram_tensor("w1", [4, 128], F32, kind="ExternalInput")
    w2_d = nc.dram_tensor("w2", [128, 128], F32, kind="ExternalInput")
    wl_d = nc.dram_tensor("wl", [128, OUT], F32, kind="ExternalInput")
    ivd_d = nc.dram_tensor("ivd", [128, NT], F32, kind="ExternalInput")
    bcol_d = nc.dram_tensor("bcol", [128, NT], F32, kind="ExternalInput")
    wvc_d = nc.dram_tensor("wvc", [128, NT], F32, kind="ExternalInput")
    if has_b1:
        b1_d = nc.dram_tensor("b1r", [1, 128], F32, kind="ExternalInput")
    if has_b2:
        b2_d = nc.dram_tensor("b2r", [1, 128], F32, kind="ExternalInput")
    if has_b1 or has_b2:
        sqd_d = nc.dram_tensor("sqd", [1, NPAD], F32, kind="ExternalInput")
    out_d = nc.dram_tensor("out", [G, OUT], F32, kind="ExternalOutput")

    relu = mybir.ActivationFunctionType.Relu
    copy_fn = mybir.ActivationFunctionType.Copy
    iseq = mybir.AluOpType.is_equal
    mult = mybir.AluOpType.mult

    waves = [list(range(w * WT, min((w + 1) * WT, NT))) for w in range(NW)]

    with tile.TileContext(nc) as tc:
        nc.gpsimd.load_library(mlp)
        with ExitStack() as ctx:
            const = ctx.enter_context(tc.tile_pool(name="const", bufs=1))
            dram = ctx.enter_context(tc.tile_pool(name="dram", bufs=1,
                                                  space="DRAM"))

            h1b_qt = [dram.tile([QROWS[q], 128], FP16, name=f"h1b{q}")
                      for q in range(4)]
            h1full_qt = [dram.tile([QROWS[q] * NC, 128], FP16,
                                   addr_space="Shared", name=f"h1f{q}")
                         for q in range(4)]

            # ---- constants
            idx_t = const.tile([128, TOTCH2 * 8], I16)
            issuers = [nc.sync, nc.scalar]
            for k in range(8):
                issuers[k % 2].dma_start(idx_t[16 * k:16 * k + 16, :],
                                         idx_d[:, :])
            dstw2_t = const.tile([128, TOTCH2], FP16)
            nc.sync.dma_start(dstw2_t[:], dstw2_d[:])

            ig16 = const.tile([128, 128], I16)
            nc.gpsimd.iota(ig16[:], [[1, 128]], channel_multiplier=0)
            iota_t = const.tile([128, 128], FP16)
            nc.any.tensor_copy(iota_t[:], ig16[:])
            ig256 = const.tile([128, 256], I16)
            nc.gpsimd.iota(ig256[:], [[1, 256]], channel_multiplier=0)
            iota2_t = const.tile([128, 256], FP16)
            nc.any.tensor_copy(iota2_t[:], ig256[:])
            pid16 = const.tile([128, 1], I16)
            nc.gpsimd.iota(pid16[:], [[1, 1]], channel_multiplier=1)
            pidf = const.tile([128, 1], F32)
            nc.any.tensor_copy(pidf[:], pid16[:])
            ident_t = const.tile([128, 128], F32)
            nc.vector.tensor_scalar(ident_t[:], ig16[:], pidf[:], None, iseq)
            identb_t = const.tile([128, 128], FP16)
            nc.any.tensor_copy(identb_t[:], ident_t[:])
            iotag_t = const.tile([128, G], F32)
            nc.any.tensor_copy(iotag_t[:], ig16[:, 0:G])
            zc_t = const.tile([1, 512], FP16)
            nc.vector.memset(zc_t[:], 0.0)

            w1_t = const.tile([4, 128], F32)
            nc.sync.dma_start(w1_t[:], w1_d[:])
            w2_t = const.tile([128, 128], F32)
            nc.sync.dma_start(w2_t[:], w2_d[:])
            wl_t = const.tile([128, OUT], F32)
            nc.sync.dma_start(wl_t[:], wl_d[:])
            ivd_t = const.tile([128, NT], F32)
            nc.sync.dma_start(ivd_t[:], ivd_d[:])
            bcol_t = const.tile([128, NT], F32)
            nc.sync.dma_start(bcol_t[:], bcol_d[:])
            wvc_t = const.tile([128, NT], F32)
            nc.sync.dma_start(wvc_t[:], wvc_d[:])
            if has_b1:
                b1r_t = const.tile([1, 128], F32)
                nc.sync.dma_start(b1r_t[:], b1_d[:])
            if has_b2:
                b2r_t = const.tile([1, 128], F32)
                nc.sync.dma_start(b2r_t[:], b2_d[:])
            if has_b1 or has_b2:
                sqd_t = const.tile([1, NPAD], F32)
                nc.sync.dma_start(sqd_t[:], sqd_d[:])
            h1sb = const.tile([128, NPAD], FP16)

            # ---- pools
            m1p = ctx.enter_context(tc.tile_pool(name="m1", bufs=2))
            mwp = ctx.enter_context(tc.tile_pool(name="mw", bufs=5))
            ohp = ctx.enter_context(tc.tile_pool(name="oh", bufs=2))
            asbp = ctx.enter_context(tc.tile_pool(name="asb", bufs=2))
            rlp = ctx.enter_context(tc.tile_pool(name="rl", bufs=2))
            stp = ctx.enter_context(tc.tile_pool(name="st", bufs=2))
            pwp = ctx.enter_context(tc.tile_pool(name="pw", bufs=1))
            p512 = ctx.enter_context(tc.tile_pool(name="p512", bufs=4,
                                                  space="PSUM"))
            trp = ctx.enter_context(tc.tile_pool(name="tr", bufs=2,
                                                 space="PSUM"))
            p2p = ctx.enter_context(tc.tile_pool(name="p2", bufs=1,
                                                 space="PSUM"))
            plp = ctx.enter_context(tc.tile_pool(name="pl", bufs=1,
                                                 space="PSUM"))

            # ================= layer 1 =================
            for w, wt in enumerate(waves):
                nw_ = len(wt)
                kw = K_w[w]
                ncols = nw_ * 4 * kw
                msgw = m1p.tile([128, M1WMAX], FP16, tag="m1")
                nc.sync.dma_start(msgw[:, 0:ncols],
                                  msg1_d[:, int(mb1[w]):int(mb1[w]) + ncols])
                aggw = asbp.tile([128, WT * 4], F32, tag="agg")
                nc.vector.tensor_reduce(
                    aggw[:, 0:nw_ * 4].rearrange("p (t f) -> p t f", f=4),
                    msgw[:, 0:ncols].rearrange("p (t f k) -> p t f k",
                                               f=4, k=kw),
                    mybir.AxisListType.X, mybir.AluOpType.add)
                # transpose agg to [4, dst] halves, then one W1 mm per half
                nhalf = (nw_ + 3) // 4
                at_ps = [p512.tile([128, 512], F32, tag="p512",
                                   name=f"at_w{w}_{h}")
                         for h in range(nhalf)]
                for j, t in enumerate(wt):
                    nc.tensor.matmul(
                        at_ps[j // 4][0:4, (j % 4) * 128:(j % 4 + 1) * 128],
                        aggw[:, j * 4:j * 4 + 4], ident_t[:],
                        start=True, stop=True, skip_group_check=True)
                for h in range(nhalf):
                    ncol_h = min(512, (nw_ - h * 4) * 128)
                    at_sb = stp.tile([4, 512], F32, tag="at")
                    nc.scalar.activation(at_sb[0:4, 0:ncol_h],
                                         at_ps[h][0:4, 0:ncol_h], copy_fn)
                    p2w = p512.tile([128, 512], F32, tag="p512",
                                    name=f"p2w_w{w}_{h}")
                    nc.tensor.matmul(p2w[:, 0:ncol_h], w1_t[:],
                                     at_sb[0:4, 0:ncol_h],
                                     start=True, stop=not has_b1,
                                     skip_group_check=True)
                    if has_b1:
                        c0 = (wt[0] + h * 4) * 128
                        nc.tensor.matmul(p2w[:, 0:ncol_h], b1r_t[:],
                                         sqd_t[0:1, c0:c0 + ncol_h],
                                         start=False, stop=True,
                                         skip_group_check=True)
                    rl_sb = rlp.tile([128, 512], F32, tag="rl")
                    nc.scalar.activation(rl_sb[:, 0:ncol_h],
                                         p2w[:, 0:ncol_h], relu)
                    for j in range(h * 4, min(nw_, h * 4 + 4)):
                        t = wt[j]
                        tnm = trp.tile([128, 128], F32, tag="tr")
                        nc.tensor.matmul(
                            tnm[:], rl_sb[:, (j % 4) * 128:(j % 4 + 1) * 128],
                            ident_t[:], start=True, stop=True,
                            skip_group_check=True)
                        nc.scalar.activation(
                            h1sb[:, t * 128:t * 128 + 128], tnm[:],
                            copy_fn, scale=ivd_t[:, t:t + 1])
                # bounce to the quarter's DRAM buffer
                q = next(qi for qi, (w0, w1) in enumerate(QWAVES)
                         if w0 <= w < w1)
                qbase = cfg.QSTART[q] * 128
                base = wt[0] * 128
                nfull = sum(1 for t in wt if (t + 1) * 128 <= NPC)
                if nfull:
                    nc.sync.dma_start(
                        h1b_qt[q][base - qbase:base - qbase + nfull * 128, :]
                        .rearrange("(j p) f -> p j f", p=128),
                        h1sb[:, base:base + nfull * 128]
                        .rearrange("p (j f) -> p j f", f=128))
                for t in wt:
                    if (t + 1) * 128 <= NPC:
                        continue
                    rows = NPC - t * 128
                    if rows > 0:
                        nc.sync.dma_start(
                            h1b_qt[q][t * 128 - qbase:
                                      t * 128 - qbase + rows, :],
                            h1sb[0:rows, t * 128:(t + 1) * 128])
                for qi, (w0, w1) in enumerate(QWAVES):
                    if w == w1 - 1:
                        nc.gpsimd.collective_compute(
                            "AllGather", mybir.AluOpType.bypass,
                            replica_groups=GROUPS,
                            ins=[h1b_qt[qi][:].opt()],
                            outs=[h1full_qt[qi][:].opt()])

            # ================= layer 2 =================
            pooled_ps = plp.tile([128, G], F32)
            gq = 0
            for w, wt in enumerate(waves):
                msgs = {}
                for q in range(4):
                    c0, nch = wq_meta[w][q]
                    if nch == 0:
                        continue
                    ni = nch * 128
                    mq = mwp.tile([128, CWQ, 128], FP16, tag="mw")
                    nc.gpsimd.dma_gather(
                        mq[:, 0:nch, :],
                        h1full_qt[q][:, :],
                        idx_t[:, c0 * 8:c0 * 8 + ni // 16],
                        ni, ni, 128, single_packet=False,
                        queue_num=gq % 4)
                    gq += 1
                    msgs[q] = mq
                aggs = [p512.tile([128, 512], F32, tag="p512",
                                  name=f"agg2_w{w}_{h}")
                        for h in range((len(wt) + 3) // 4)]
                for agg in aggs:
                    nc.tensor.matmul(agg[:], zc_t[0:1, 0:128],
                                     zc_t[0:1, 0:512],
                                     start=True, stop=False,
                                     skip_group_check=True)
                pwt = pwp.tile([128, 2, WT, G], F32, tag="pw")
                nw_ = len(wt)
                nc.vector.tensor_tensor(
                    pwt[:, 0, 0:nw_, :],
                    bcol_t[:, wt[0]:wt[0] + nw_].unsqueeze(2)
                    .broadcast_to([128, nw_, G]),
                    iotag_t[:, :].unsqueeze(1).broadcast_to([128, nw_, G]),
                    iseq)
                nc.vector.tensor_tensor(
                    pwt[:, 1, 0:nw_, :], pwt[:, 0, 0:nw_, :],
                    wvc_t[:, wt[0]:wt[0] + nw_].unsqueeze(2)
                    .broadcast_to([128, nw_, G]),
                    mult)
                wpairs = list(range(wt[0] // 2, (wt[-1] + 2) // 2))
                for q in range(4):
                    c0, nch = wq_meta[w][q]
                    if nch == 0:
                        continue
                    mq = msgs[q]
                    oh = ohp.tile([128, CWQ, 256], FP16, tag="oh")
                    nc.vector.tensor_tensor(
                        oh[:, 0:nch, :],
                        dstw2_t[:, c0:c0 + nch].unsqueeze(2)
                        .broadcast_to([128, nch, 256]),
                        iota2_t[:, :].unsqueeze(1)
                        .broadcast_to([128, nch, 256]),
                        iseq)
                    for jp, pp in enumerate(wpairs):
                        psl = aggs[jp // 2][:, (jp % 2) * 256:
                                            (jp % 2) * 256 + 256]
                        for k in range(int(nch2[pp, q])):
                            cc = int(cb2[pp, q]) + k
                            nc.tensor.matmul(
                                psl, mq[:, cc - c0, :], oh[:, cc - c0, :],
                                start=False, stop=False,
                                skip_group_check=True)
                for j, t in enumerate(wt):
                    rows = min(128, NPC - t * 128)
                    psl = aggs[j // 4][:, (j % 4) * 128:(j % 4) * 128 + 128]
                    nc.tensor.matmul(
                        psl, h1sb[0:rows, t * 128:t * 128 + 128],
                        identb_t[0:rows, :],
                        start=False,
                        stop=(j % 4 == 3 or j == len(wt) - 1),
                        skip_group_check=True)
                for j, t in enumerate(wt):
                    psl = aggs[j // 4][:, (j % 4) * 128:(j % 4) * 128 + 128]
                    agg_sb = asbp.tile([128, 128], F32, tag="asb2")
                    nc.scalar.activation(agg_sb[:], psl, copy_fn)
                    p2 = p2p.tile([128, 128], F32, tag="p2")
                    nc.tensor.matmul(p2[:], w2_t[:], agg_sb[:],
                                     start=True, stop=not has_b2)
                    if has_b2:
                        nc.tensor.matmul(p2[:], b2r_t[:],
                                         sqd_t[0:1, t * 128:t * 128 + 128],
                                         start=False, stop=True)
                    relu_sb = rlp.tile([128, 128], F32, tag="rl2")
                    nc.scalar.activation(relu_sb[:], p2[:], relu)
                    tnm = trp.tile([128, 128], F32, tag="tr")
                    nc.tensor.matmul(tnm[:], relu_sb[:], ident_t[:],
                                     start=True, stop=True,
                                     skip_group_check=True)
                    tnm_sb = stp.tile([128, 128], F32, tag="tnm")
                    nc.scalar.activation(tnm_sb[:], tnm[:], copy_fn)
                    nc.tensor.matmul(
                        pooled_ps[:], tnm_sb[:], pwt[:, 1, j, :],
                        start=(t == 0), stop=(t == NT - 1),
                        skip_group_check=True)

            pooled_sb = const.tile([128, G], F32)
            nc.any.tensor_copy(pooled_sb[:], pooled_ps[:])
            outp = p2p.tile([128, 128], F32, tag="p2")
            nc.tensor.matmul(outp[0:G, 0:OUT], pooled_sb[:], wl_t[:],
                             start=True, stop=True, skip_group_check=True)
            out_sb = const.tile([G, OUT], F32)
            nc.any.tensor_copy(out_sb[:], outp[0:G, 0:OUT])
            nc.sync.dma_start(out_d[:], out_sb[:])

    nc.compile()
    return nc


# --------------------------------------------------------------------------
# driver
# --------------------------------------------------------------------------

def _run(cfg, meta, W1, b1, W2, b2, Wl, bl, runner):
    NC = cfg.NCORES
    has_b1 = bool(np.any(np.asarray(b1)))
    has_b2 = bool(np.any(np.asarray(b2)))

    assert cfg.DIN <= 4
    W1p = np.zeros((4, 128), dtype=np.float32)
    W1p[:cfg.DIN] = np.asarray(W1, dtype=np.float32)

    nc = build_fused(cfg, meta, has_b1, has_b2)
    in_maps = []
    for c in range(NC):
        m = dict(
            msg1=meta["msg1"][c], idx=meta["idx16"][c],
            dstw2=meta["dstw2"][c],
            w1=W1p, w2=np.asarray(W2, np.float32),
            wl=np.asarray(Wl, np.float32),
            ivd=meta["ivd"][c], bcol=meta["bcol"][c], wvc=meta["wvc"][c],
        )
        if has_b1:
            m["b1r"] = np.asarray(b1, np.float32).reshape(1, 128)
        if has_b2:
            m["b2r"] = np.asarray(b2, np.float32).reshape(1, 128)
        if has_b1 or has_b2:
            m["sqd"] = meta["sqd"][c]
        in_maps.append(m)
    res = runner(nc, in_maps)
    total = np.sum([res[c]["out"] for c in range(NC)], axis=0)
    return (total + np.asarray(bl, np.float32)[None, :]).astype(np.float32)


def _hw_runner(nc, in_maps):
    core_ids = list(range(len(in_maps)))
    try:
        res = run_bass_kernel_spmd(nc, in_maps, core_ids=core_ids)
    except Exception:
        res = run_bass_kernel_spmd(nc, in_maps, core_ids=core_ids)
    return res.results


def kernel(x, edge_index, batch, W1, b1, W2, b2, Wl, bl):
    cfg = FULL
    meta = preprocess(cfg, x, edge_index, batch)
    return _run(cfg, meta, W1, b1, W2, b2, Wl, bl, _hw_runner)


# revision 3
# speedup vs baseline: 1.1046x; 1.0296x over previous
"""GCN (2x GCNConv + mean-pool + linear) on 8 Trainium2 NeuronCores.

On-device time is bound by the layer-2 per-edge dma_gather (256B rows,
~55GB/s SWDGE ceiling), so v4 makes everything else nearly free and
overlaps it under the gather window:

  - Nodes are assigned to (tile, partition) slots per core by descending
    in-degree (the node->slot map is free to choose; all per-node tables
    are host-built in slot order). Layer-1 messages are then shipped as a
    dense dst-major table [128, wave, tile, feat, k] so the whole layer-1
    aggregation is a single DVE tensor_reduce per wave: no gather, no
    one-hots, no scatter matmuls; the self-loop is just one more entry.
  - Layer-1 epilogue is wave-batched: per-tile PE transposes into PSUM,
    one W1 matmul per 512-column half, one relu per half.
  - Layer-1 output is AllGather'd in 4 wave-aligned quarters overlapping
    layer-1; layer-2 gathers (tile-pure 128-slot chunks) start as soon as
    their quarter table lands and run continuously on all 4 SWDGE queues
    into wave-sized message tiles.
  - Layer-2 scatter one-hots are built in ONE DVE tensor_tensor(is_equal)
    with broadcast APs per (wave, quarter) group; h1 tables, messages and
    one-hots are fp8e4m3 (halves PE time; ~1e-3 output error, budget 2e-2).
"""

import sys
from contextlib import ExitStack

for _p in ("/opt/trn_rl_repo",):
    if _p not in sys.path:
        sys.path.insert(0, _p)

import numpy as np

import concourse.bass as bass
import concourse.mybir as mybir
import concourse.tile as tile
from concourse import bacc
from concourse.bass_utils import run_bass_kernel_spmd
from concourse.library_config import mlp

FP16 = mybir.dt.float16
F32 = mybir.dt.float32
I16 = mybir.dt.int16
FP8 = mybir.dt.float8e4
FP16_NP = np.float16
SENT = 16384.0


class Cfg:
    def __init__(self, N=100000, E=1600000, G=100, DIN=3, H=128, OUT=10,
                 NCORES=8, WT=8):
        self.N, self.E, self.G = N, E, G
        self.DIN, self.H, self.OUT = DIN, H, OUT
        self.NCORES = NCORES
        assert N % NCORES == 0
        self.NPC = N // NCORES
        self.NT = (self.NPC + 127) // 128
        self.WT = WT
        self.NW = (self.NT + WT - 1) // WT
        self.QWAVES = [(0, 3), (3, 6), (6, 9), (9, self.NW)]
        self.QSTART = [w0 * WT for (w0, _) in self.QWAVES]
        qend = [min(w1 * WT, self.NT) for (_, w1) in self.QWAVES]
        self.QROWS = [min(e * 128, self.NPC) - s * 128
                      for s, e in zip(self.QSTART, qend)]
        assert sum(self.QROWS) == self.NPC
        assert all(r * NCORES < 32768 for r in self.QROWS)


FULL = Cfg()


# --------------------------------------------------------------------------
# host preprocessing
# --------------------------------------------------------------------------

def preprocess(cfg, x, edge_index, batch):
    N, G, NC = cfg.N, cfg.G, cfg.NCORES
    NPC, NT, WT, NW = cfg.NPC, cfg.NT, cfg.WT, cfg.NW
    src = np.asarray(edge_index[0], dtype=np.int64)
    dst = np.asarray(edge_index[1], dtype=np.int64)
    batch = np.asarray(batch, dtype=np.int64)
    x = np.asarray(x, dtype=np.float32)
    E = src.shape[0]

    deg = (np.bincount(dst, minlength=N) + 1.0).astype(np.float32)
    dinsq = (1.0 / np.sqrt(deg)).astype(np.float32)
    sqrtdeg = np.sqrt(deg).astype(np.float32)
    cnt = np.bincount(batch, minlength=G).astype(np.float32)
    invcnt = (1.0 / np.maximum(cnt, 1.0)).astype(np.float32)

    xs16 = np.zeros((N, 4), dtype=FP16_NP)
    xs16[:, :cfg.DIN] = (x * dinsq[:, None]).astype(FP16_NP)

    # ---- per-core node -> slot map (identity; degree sorting concentrates
    # L2 edges into early waves and blows up the message tiles)
    order = np.tile(np.arange(NPC), (NC, 1))           # slot -> local node
    slotof = order                                     # local node -> slot

    core = dst // NPC
    dstl = dst - core * NPC
    s_dst = slotof[core, dstl]
    t = s_dst >> 7
    dit = (s_dst & 127).astype(np.float32)

    # ---- L1 dst-major message table with per-wave entry budget K_w
    dcnt = np.zeros((NC, NT * 128), np.int64)          # per-slot in-degree
    np.add.at(dcnt, (core, s_dst), 1)
    K_w = []
    for w in range(NW):
        lo, hi = w * WT * 128, min((w + 1) * WT, NT) * 128
        K_w.append(int(dcnt[:, lo:hi].max()) + 1)      # +1 self entry
    mb1 = np.zeros(NW + 1, np.int64)                   # col base per wave
    for w in range(NW):
        nw_ = min((w + 1) * WT, NT) - w * WT
        mb1[w + 1] = mb1[w] + nw_ * 4 * K_w[w]
    M1COLS = int(mb1[NW])

    msg1 = np.zeros((NC, 128, M1COLS), dtype=FP16_NP)
    # edge entries: k = rank within (core, dst-slot)
    okey = core * (NT * 128) + s_dst
    oo = np.argsort(okey, kind="stable")
    bs = np.zeros(NC * NT * 128, np.int64)
    bs[1:] = np.cumsum(dcnt.reshape(-1))[:-1]
    krank = np.arange(E) - bs[okey[oo]]
    s_o, c_o = s_dst[oo], core[oo]
    t_o = s_o >> 7
    w_o = t_o // WT
    j_o = t_o - w_o * WT
    p_o = s_o & 127
    kw_o = np.array(K_w)[w_o]
    colbase = mb1[w_o] + (j_o * 4) * kw_o + krank
    vals = xs16[src[oo]]
    for f in range(4):
        msg1[c_o, p_o, colbase + f * kw_o] = vals[:, f]
    # self entries at k = dcnt
    for c in range(NC):
        sarr = np.arange(NT * 128)
        valid = sarr < NPC
        sv = sarr[valid]
        tv = sv >> 7
        wv = tv // WT
        jv = tv - wv * WT
        pv = sv & 127
        kwv = np.array(K_w)[wv]
        cb = mb1[wv] + (jv * 4) * kwv + dcnt[c, sv]
        xvals = xs16[c * NPC + order[c][sv]]
        for f in range(4):
            msg1[c, pv, cb + f * kwv] = xvals[:, f]

    # ---- L2 slot layout: tile-pure chunks grouped (wave, quarter, tile)
    QROWS, QSTART = np.array(cfg.QROWS), np.array(cfg.QSTART)
    srcc = src // NPC
    srcl = src - srcc * NPC
    s_src = slotof[srcc, srcl]
    srct = s_src >> 7
    srcq = np.searchsorted(np.cumsum(QROWS), s_src, side="right")
    toff = srcc * QROWS[srcq] + (s_src - QSTART[srcq] * 128)
    assert toff.max() < 32768

    # chunks are tile-PAIR-pure: each 128-slot chunk targets one pair of
    # adjacent tiles (one-hot window 256, one matmul per chunk)
    NPR = (NT + 1) // 2
    pr = t >> 1
    dpr = (s_dst & 255).astype(np.float32)     # dst rel to pair start
    cnt2 = np.zeros((NC, NPR, 4), np.int64)
    np.add.at(cnt2, (core, pr, srcq), 1)
    nch2 = (cnt2.max(axis=0) + 127) // 128     # [NPR, 4]
    cb2 = np.zeros((NPR, 4), np.int64)
    pos = 0
    wq_meta = []
    for w in range(NW):
        wpairs = range(w * WT // 2, (min((w + 1) * WT, NT) + 1) // 2)
        wm = []
        for q in range(4):
            c0 = pos
            for pp in wpairs:
                cb2[pp, q] = pos
                pos += int(nch2[pp, q])
            wm.append((c0, pos - c0))
        wq_meta.append(wm)
    TOTCH2 = pos

    key2 = (core * NPR + pr) * 4 + srcq
    order2 = np.argsort(key2, kind="stable")
    b2s = np.zeros(NC * NPR * 4, np.int64)
    b2s[1:] = np.cumsum(cnt2.reshape(-1))[:-1]
    rank2 = np.arange(E) - b2s[key2[order2]]
    slot2 = cb2[pr[order2], srcq[order2]] * 128 + rank2
    c_o2 = core[order2]
    p2_, cc2 = slot2 % 128, slot2 // 128

    idxf = np.zeros((NC, TOTCH2 * 128), dtype=np.int16)
    dstw2 = np.full((NC, 128, TOTCH2), SENT, dtype=FP16_NP)
    idxf[c_o2, slot2] = toff[order2].astype(np.int16)
    dstw2[c_o2, p2_, cc2] = dpr[order2].astype(FP16_NP)
    idx16 = np.ascontiguousarray(
        idxf.reshape(NC, TOTCH2 * 8, 16).transpose(0, 2, 1))

    # ---- per-slot epilogue columns (slot order!)
    NPAD = NT * 128
    sarr = np.arange(NPAD)
    valid = sarr < NPC
    sv = np.where(valid, sarr, 0)
    ivd = np.zeros((NC, 128, NT), np.float32)
    bcolv = np.zeros((NC, 128, NT), np.float32)
    wvc = np.zeros((NC, 128, NT), np.float32)
    sqd = np.ones((NC, 1, NPAD), np.float32)
    for c in range(NC):
        g = c * NPC + order[c][sv]
        ivd[c] = np.where(valid, dinsq[g] ** 2, 1.0).reshape(NT, 128).T
        bcolv[c] = np.where(valid, batch[g].astype(np.float32),
                            SENT).reshape(NT, 128).T
        wvc[c] = np.where(valid, dinsq[g] * invcnt[batch[g]],
                          0.0).reshape(NT, 128).T
        sqd[c, 0] = np.where(valid, sqrtdeg[g], 1.0)

    CW2 = max(sum(n for (_, n) in wm) for wm in wq_meta)
    CWQ = max(n for wm in wq_meta for (_, n) in wm)

    return dict(
        msg1=msg1, idx16=idx16, dstw2=dstw2,
        ivd=ivd, bcol=bcolv, wvc=wvc, sqd=sqd,
        nch2=nch2, cb2=cb2, TOTCH2=TOTCH2, wq_meta=wq_meta,
        K_w=K_w, mb1=mb1, M1COLS=M1COLS, CW2=CW2, CWQ=CWQ, deg=deg,
    )


# --------------------------------------------------------------------------
# fused kernel builder
# --------------------------------------------------------------------------

def build_fused(cfg, meta, has_b1, has_b2):
    N, G, OUT = cfg.N, cfg.G, cfg.OUT
    NT, WT, NPC, NW = cfg.NT, cfg.WT, cfg.NPC, cfg.NW
    NPAD = NT * 128
    NC = cfg.NCORES
    GROUPS = [list(range(NC))]
    TOTCH2 = meta["TOTCH2"]
    nch2, cb2 = meta["nch2"], meta["cb2"]
    wq_meta = meta["wq_meta"]
    K_w, mb1, M1COLS = meta["K_w"], meta["mb1"], meta["M1COLS"]
    CW2, CWQ = meta["CW2"], meta["CWQ"]
    QROWS, QWAVES = cfg.QROWS, cfg.QWAVES
    M1WMAX = max(mb1[w + 1] - mb1[w] for w in range(NW))

    nc = bacc.Bacc("TRN2", target_bir_lowering=False, debug=False,
                   num_devices=NC, num_swdge_queues=4,
                   dynamic_dma_scratch_size=16384)
    msg1_d = nc.dram_tensor("msg1", [128, M1COLS], FP16,
                            kind="ExternalInput")
    idx_d = nc.dram_tensor("idx", [16, TOTCH2 * 8], I16,
                           kind="ExternalInput")
    dstw2_d = nc.dram_tensor("dstw2", [128, TOTCH2], FP16,
                             kind="ExternalInput")
    w1_d = nc.dram_tensor("w1", [4, 128], F32, kind="ExternalInput")
    w2_d = nc.dram_tensor("w2", [128, 128], F32, kind="ExternalInput")
    wl_d = nc.dram_tensor("wl", [128, OUT], F32, kind="ExternalInput")
    ivd_d = nc.dram_tensor("ivd", [128, NT], F32, kind="ExternalInput")
    bcol_d = nc.dram_tensor("bcol", [128, NT], F32, kind="ExternalInput")
    wvc_d = nc.dram_tensor("wvc", [128, NT], F32, kind="ExternalInput")
    if has_b1:
        b1_d = nc.dram_tensor("b1r", [1, 128], F32, kind="ExternalInput")
    if has_b2:
        b2_d = nc.dram_tensor("b2r", [1, 128], F32, kind="ExternalInput")
    if has_b1 or has_b2:
        sqd_d = nc.dram_tensor("sqd", [1, NPAD], F32, kind="ExternalInput")
    out_d = nc.dram_tensor("out", [G, OUT], F32, kind="ExternalOutput")

    relu = mybir.ActivationFunctionType.Relu
    copy_fn = mybir.ActivationFunctionType.Copy
    iseq = mybir.AluOpType.is_equal
    mult = mybir.AluOpType.mult

    waves = [list(range(w * WT, min((w + 1) * WT, NT))) for w in range(NW)]

    with tile.TileContext(nc) as tc:
        nc.gpsimd.load_library(mlp)
        with ExitStack() as ctx:
            const = ctx.enter_context(tc.tile_pool(name="const", bufs=1))
            dram = ctx.enter_context(tc.tile_pool(name="dram", bufs=1,
                                                  space="DRAM"))

            h1b_qt = [dram.tile([QROWS[q], 256], FP8, name=f"h1b{q}")
                      for q in range(4)]
            h1full_qt = [dram.tile([QROWS[q] * NC, 256], FP8,
                                   addr_space="Shared", name=f"h1f{q}")
                         for q in range(4)]

            # ---- constants
            idx_t = const.tile([128, TOTCH2 * 8], I16)
            issuers = [nc.sync, nc.scalar]
            for k in range(8):
                issuers[k % 2].dma_start(idx_t[16 * k:16 * k + 16, :],
                                         idx_d[:, :])
            dstw2_t = const.tile([128, TOTCH2], FP16)
            nc.sync.dma_start(dstw2_t[:], dstw2_d[:])

            ig16 = const.tile([128, 128], I16)
            nc.gpsimd.iota(ig16[:], [[1, 128]], channel_multiplier=0)
            iota_t = const.tile([128, 128], FP16)
            nc.any.tensor_copy(iota_t[:], ig16[:])
            ig256 = const.tile([128, 256], I16)
            nc.gpsimd.iota(ig256[:], [[1, 256]], channel_multiplier=0)
            iota2_t = const.tile([128, 256], FP16)
            nc.any.tensor_copy(iota2_t[:], ig256[:])
            pid16 = const.tile([128, 1], I16)
            nc.gpsimd.iota(pid16[:], [[1, 1]], channel_multiplier=1)
            pidf = const.tile([128, 1], F32)
            nc.any.tensor_copy(pidf[:], pid16[:])
            ident_t = const.tile([128, 128], F32)
            nc.vector.tensor_scalar(ident_t[:], ig16[:], pidf[:], None, iseq)
            identb_t = const.tile([128, 128], FP8)
            nc.any.tensor_copy(identb_t[:], ident_t[:])
            iotag_t = const.tile([128, G], F32)
            nc.any.tensor_copy(iotag_t[:], ig16[:, 0:G])
            zc_t = const.tile([1, 512], FP8)
            nc.vector.memset(zc_t[:], 0.0)

            w1_t = const.tile([4, 128], F32)
            nc.sync.dma_start(w1_t[:], w1_d[:])
            w2_t = const.tile([128, 128], F32)
            nc.sync.dma_start(w2_t[:], w2_d[:])
            wl_t = const.tile([128, OUT], F32)
            nc.sync.dma_start(wl_t[:], wl_d[:])
            ivd_t = const.tile([128, NT], F32)
            nc.sync.dma_start(ivd_t[:], ivd_d[:])
            bcol_t = const.tile([128, NT], F32)
            nc.sync.dma_start(bcol_t[:], bcol_d[:])
            wvc_t = const.tile([128, NT], F32)
            nc.sync.dma_start(wvc_t[:], wvc_d[:])
            if has_b1:
                b1r_t = const.tile([1, 128], F32)
                nc.sync.dma_start(b1r_t[:], b1_d[:])
            if has_b2:
                b2r_t = const.tile([1, 128], F32)
                nc.sync.dma_start(b2r_t[:], b2_d[:])
            if has_b1 or has_b2:
                sqd_t = const.tile([1, NPAD], F32)
                nc.sync.dma_start(sqd_t[:], sqd_d[:])
            h1sb = const.tile([128, NPAD], FP8)

            # ---- pools
            m1p = ctx.enter_context(tc.tile_pool(name="m1", bufs=2))
            mwp = ctx.enter_context(tc.tile_pool(name="mw", bufs=7))
            ohp = ctx.enter_context(tc.tile_pool(name="oh", bufs=3))
            asbp = ctx.enter_context(tc.tile_pool(name="asb", bufs=2))
            rlp = ctx.enter_context(tc.tile_pool(name="rl", bufs=2))
            stp = ctx.enter_context(tc.tile_pool(name="st", bufs=2))
            pwp = ctx.enter_context(tc.tile_pool(name="pw", bufs=1))
            p512 = ctx.enter_context(tc.tile_pool(name="p512", bufs=4,
                                                  space="PSUM"))
            trp = ctx.enter_context(tc.tile_pool(name="tr", bufs=2,
                                                 space="PSUM"))
            p2p = ctx.enter_context(tc.tile_pool(name="p2", bufs=1,
                                                 space="PSUM"))
            plp = ctx.enter_context(tc.tile_pool(name="pl", bufs=1,
                                                 space="PSUM"))

            # ================= layer 1 =================
            for w, wt in enumerate(waves):
                nw_ = len(wt)
                kw = K_w[w]
                ncols = nw_ * 4 * kw
                msgw = m1p.tile([128, M1WMAX], FP16, tag="m1")
                nc.sync.dma_start(msgw[:, 0:ncols],
                                  msg1_d[:, int(mb1[w]):int(mb1[w]) + ncols])
                aggw = asbp.tile([128, WT * 4], F32, tag="agg")
                nc.vector.tensor_reduce(
                    aggw[:, 0:nw_ * 4].rearrange("p (t f) -> p t f", f=4),
                    msgw[:, 0:ncols].rearrange("p (t f k) -> p t f k",
                                               f=4, k=kw),
                    mybir.AxisListType.X, mybir.AluOpType.add)
                # transpose agg to [4, dst] halves, then one W1 mm per half
                nhalf = (nw_ + 3) // 4
                at_ps = [p512.tile([128, 512], F32, tag="p512",
                                   name=f"at_w{w}_{h}")
                         for h in range(nhalf)]
                for j, t in enumerate(wt):
                    nc.tensor.matmul(
                        at_ps[j // 4][0:4, (j % 4) * 128:(j % 4 + 1) * 128],
                        aggw[:, j * 4:j * 4 + 4], ident_t[:],
                        start=True, stop=True, skip_group_check=True)
                for h in range(nhalf):
                    ncol_h = min(512, (nw_ - h * 4) * 128)
                    at_sb = stp.tile([4, 512], F32, tag="at")
                    nc.scalar.activation(at_sb[0:4, 0:ncol_h],
                                         at_ps[h][0:4, 0:ncol_h], copy_fn)
                    p2w = p512.tile([128, 512], F32, tag="p512",
                                    name=f"p2w_w{w}_{h}")
                    nc.tensor.matmul(p2w[:, 0:ncol_h], w1_t[:],
                                     at_sb[0:4, 0:ncol_h],
                                     start=True, stop=not has_b1,
                                     skip_group_check=True)
                    if has_b1:
                        c0 = (wt[0] + h * 4) * 128
                        nc.tensor.matmul(p2w[:, 0:ncol_h], b1r_t[:],
                                         sqd_t[0:1, c0:c0 + ncol_h],
                                         start=False, stop=True,
                                         skip_group_check=True)
                    rl_sb = rlp.tile([128, 512], F32, tag="rl")
                    nc.scalar.activation(rl_sb[:, 0:ncol_h],
                                         p2w[:, 0:ncol_h], relu)
                    for j in range(h * 4, min(nw_, h * 4 + 4)):
                        t = wt[j]
                        tnm = trp.tile([128, 128], F32, tag="tr")
                        nc.tensor.matmul(
                            tnm[:], rl_sb[:, (j % 4) * 128:(j % 4 + 1) * 128],
                            ident_t[:], start=True, stop=True,
                            skip_group_check=True)
                        nc.scalar.activation(
                            h1sb[:, t * 128:t * 128 + 128], tnm[:],
                            copy_fn, scale=ivd_t[:, t:t + 1])
                # bounce to the quarter's DRAM buffer
                q = next(qi for qi, (w0, w1) in enumerate(QWAVES)
                         if w0 <= w < w1)
                qbase = cfg.QSTART[q] * 128
                base = wt[0] * 128
                nfull = sum(1 for t in wt if (t + 1) * 128 <= NPC)
                if nfull:
                    nc.sync.dma_start(
                        h1b_qt[q][base - qbase:base - qbase + nfull * 128, 0:128]
                        .rearrange("(j p) f -> p j f", p=128),
                        h1sb[:, base:base + nfull * 128]
                        .rearrange("p (j f) -> p j f", f=128))
                for t in wt:
                    if (t + 1) * 128 <= NPC:
                        continue
                    rows = NPC - t * 128
                    if rows > 0:
                        nc.sync.dma_start(
                            h1b_qt[q][t * 128 - qbase:
                                      t * 128 - qbase + rows, 0:128],
                            h1sb[0:rows, t * 128:(t + 1) * 128])
                for qi, (w0, w1) in enumerate(QWAVES):
                    if w == w1 - 1:
                        nc.gpsimd.collective_compute(
                            "AllGather", mybir.AluOpType.bypass,
                            replica_groups=GROUPS,
                            ins=[h1b_qt[qi][:].opt()],
                            outs=[h1full_qt[qi][:].opt()])

            # ================= layer 2 =================
            pooled_ps = plp.tile([128, G], F32)
            gq = 0
            for w, wt in enumerate(waves):
                msgs = {}
                for q in range(4):
                    c0, nch = wq_meta[w][q]
                    if nch == 0:
                        continue
                    ni = nch * 128
                    mq = mwp.tile([128, CWQ, 256], FP8, tag="mw")
                    nc.gpsimd.dma_gather(
                        mq[:, 0:nch, :],
                        h1full_qt[q][:, :],
                        idx_t[:, c0 * 8:c0 * 8 + ni // 16],
                        ni, ni, 256, single_packet=False,
                        queue_num=gq % 4)
                    gq += 1
                    msgs[q] = mq
                aggs = [p512.tile([128, 512], F32, tag="p512",
                                  name=f"agg2_w{w}_{h}")
                        for h in range((len(wt) + 3) // 4)]
                for agg in aggs:
                    nc.tensor.matmul(agg[:], zc_t[0:1, 0:128],
                                     zc_t[0:1, 0:512],
                                     start=True, stop=False,
                                     skip_group_check=True)
                pwt = pwp.tile([128, 2, WT, G], F32, tag="pw")
                nw_ = len(wt)
                nc.vector.tensor_tensor(
                    pwt[:, 0, 0:nw_, :],
                    bcol_t[:, wt[0]:wt[0] + nw_].unsqueeze(2)
                    .broadcast_to([128, nw_, G]),
                    iotag_t[:, :].unsqueeze(1).broadcast_to([128, nw_, G]),
                    iseq)
                nc.vector.tensor_tensor(
                    pwt[:, 1, 0:nw_, :], pwt[:, 0, 0:nw_, :],
                    wvc_t[:, wt[0]:wt[0] + nw_].unsqueeze(2)
                    .broadcast_to([128, nw_, G]),
                    mult)
                wpairs = list(range(wt[0] // 2, (wt[-1] + 2) // 2))
                for q in range(4):
                    c0, nch = wq_meta[w][q]
                    if nch == 0:
                        continue
                    mq = msgs[q]
                    oh = ohp.tile([128, CWQ, 256], FP8, tag="oh")
                    nc.vector.tensor_tensor(
                        oh[:, 0:nch, :],
                        dstw2_t[:, c0:c0 + nch].unsqueeze(2)
                        .broadcast_to([128, nch, 256]),
                        iota2_t[:, :].unsqueeze(1)
                        .broadcast_to([128, nch, 256]),
                        iseq)
                    for jp, pp in enumerate(wpairs):
                        psl = aggs[jp // 2][:, (jp % 2) * 256:
                                            (jp % 2) * 256 + 256]
                        for k in range(int(nch2[pp, q])):
                            cc = int(cb2[pp, q]) + k
                            nc.tensor.matmul(
                                psl, mq[:, cc - c0, 0:128], oh[:, cc - c0, :],
                                start=False, stop=False,
                                skip_group_check=True)
                for j, t in enumerate(wt):
                    rows = min(128, NPC - t * 128)
                    psl = aggs[j // 4][:, (j % 4) * 128:(j % 4) * 128 + 128]
                    nc.tensor.matmul(
                        psl, h1sb[0:rows, t * 128:t * 128 + 128],
                        identb_t[0:rows, :],
                        start=False,
                        stop=(j % 4 == 3 or j == len(wt) - 1),
                        skip_group_check=True)
                for j, t in enumerate(wt):
                    psl = aggs[j // 4][:, (j % 4) * 128:(j % 4) * 128 + 128]
                    agg_sb = asbp.tile([128, 128], F32, tag="asb2")
                    nc.scalar.activation(agg_sb[:], psl, copy_fn)
                    p2 = p2p.tile([128, 128], F32, tag="p2")
                    nc.tensor.matmul(p2[:], w2_t[:], agg_sb[:],
                                     start=True, stop=not has_b2)
                    if has_b2:
                        nc.tensor.matmul(p2[:], b2r_t[:],
                                         sqd_t[0:1, t * 128:t * 128 + 128],
                                         start=False, stop=True)
                    relu_sb = rlp.tile([128, 128], F32, tag="rl2")
                    nc.scalar.activation(relu_sb[:], p2[:], relu)
                    tnm = trp.tile([128, 128], F32, tag="tr")
                    nc.tensor.matmul(tnm[:], relu_sb[:], ident_t[:],
                                     start=True, stop=True,
                                     skip_group_check=True)
                    tnm_sb = stp.tile([128, 128], F32, tag="tnm")
                    nc.scalar.activation(tnm_sb[:], tnm[:], copy_fn)
                    nc.tensor.matmul(
                        pooled_ps[:], tnm_sb[:], pwt[:, 1, j, :],
                        start=(t == 0), stop=(t == NT - 1),
                        skip_group_check=True)

            pooled_sb = const.tile([128, G], F32)
            nc.any.tensor_copy(pooled_sb[:], pooled_ps[:])
            outp = p2p.tile([128, 128], F32, tag="p2")
            nc.tensor.matmul(outp[0:G, 0:OUT], pooled_sb[:], wl_t[:],
                             start=True, stop=True, skip_group_check=True)
            out_sb = const.tile([G, OUT], F32)
            nc.any.tensor_copy(out_sb[:], outp[0:G, 0:OUT])
            nc.sync.dma_start(out_d[:], out_sb[:])

    nc.compile()
    return nc


# --------------------------------------------------------------------------
# driver
# --------------------------------------------------------------------------

def _run(cfg, meta, W1, b1, W2, b2, Wl, bl, runner):
    NC = cfg.NCORES
    has_b1 = bool(np.any(np.asarray(b1)))
    has_b2 = bool(np.any(np.asarray(b2)))

    assert cfg.DIN <= 4
    W1p = np.zeros((4, 128), dtype=np.float32)
    W1p[:cfg.DIN] = np.asarray(W1, dtype=np.float32)

    nc = build_fused(cfg, meta, has_b1, has_b2)
    in_maps = []
    for c in range(NC):
        m = dict(
            msg1=meta["msg1"][c], idx=meta["idx16"][c],
            dstw2=meta["dstw2"][c],
            w1=W1p, w2=np.asarray(W2, np.float32),
            wl=np.asarray(Wl, np.float32),
            ivd=meta["ivd"][c], bcol=meta["bcol"][c], wvc=meta["wvc"][c],
        )
        if has_b1:
            m["b1r"] = np.asarray(b1, np.float32).reshape(1, 128)
        if has_b2:
            m["b2r"] = np.asarray(b2, np.float32).reshape(1, 128)
        if has_b1 or has_b2:
            m["sqd"] = meta["sqd"][c]
        in_maps.append(m)
    res = runner(nc, in_maps)
    total = np.sum([res[c]["out"] for c in range(NC)], axis=0)
    return (total + np.asarray(bl, np.float32)[None, :]).astype(np.float32)


def _hw_runner(nc, in_maps):
    core_ids = list(range(len(in_maps)))
    try:
        res = run_bass_kernel_spmd(nc, in_maps, core_ids=core_ids)
    except Exception:
        res = run_bass_kernel_spmd(nc, in_maps, core_ids=core_ids)
    return res.results


def kernel(x, edge_index, batch, W1, b1, W2, b2, Wl, bl):
    cfg = FULL
    meta = preprocess(cfg, x, edge_index, batch)
    return _run(cfg, meta, W1, b1, W2, b2, Wl, bl, _hw_runner)
